# revision 1
# baseline (speedup 1.0000x reference)
"""Contrastive loss (SimCLR-style NT-Xent) Trainium2 kernel.

Full inputs z1, z2: [4096, 1024] f32. Output: scalar f32 loss.

Strategy (8 NeuronCores, SPMD, no collectives):
  - Host: L2-normalize rows of reps = concat(z1, z2)  [8192, 1024] (f32),
    transpose to repsT [1024, 8192], scale by 256 and cast to fp8e4m3.
  - Row-shard the 8192x8192 similarity GEMM: core c computes rows
    [c*1024, (c+1)*1024) of sim = reps @ reps.T / T against all columns,
    using fp8 DoubleRow matmuls (K=256 per instruction, ~1.4x bf16 peak).
  - The per-core program must be identical (SPMD), but the positions of the
    self-diagonal and the positive-pair diagonal inside the row block differ
    per core. Fix: feed each core B with its *columns rotated* by
    p0 = (c*1024 + 4096) mod 8192. In rotated coordinates, for every core:
      * positive-pair entries = main diagonal of columns [0, 1024)
      * self-similarity entries = diagonal of columns [4096, 5120)
    logsumexp over a row is permutation-invariant, so nothing else changes.
  - Device, per (m-tile of 128 rows, n-chunk of 2048 cols): fp8 DoubleRow
    matmuls accumulate K=1024 in 4 instructions per 512-col PSUM bank; ACT
    exp(s*x - 10) with fused per-row accumulation (accum_out) over all 4
    banks at once; on the two special chunks copy the raw f32 logit block
    out of PSUM on ACT and diag-extract on DVE (identity mul + reduce).
  - Per row i (raw scaled dot d, pos = 10*d/SCALE^2):
      T = S_full - exp(10*dself/SCALE^2 - 10) + exp(10*draw/SCALE^2 - 10)
      loss_row = (10 + ln(T)) - (10*draw/SCALE^2)
  - Host: sum the 8192 per-row values, divide by 8192.
"""

import time
from contextlib import ExitStack

import numpy as np
import ml_dtypes

import concourse.bass as bass
import concourse.tile as tile
from concourse import bacc
from concourse import mybir
from concourse import bass_utils
from concourse.masks import make_identity

B = 4096
D = 1024
S = 2 * B  # 8192 rows/cols of sim
NCORES = 8
ROWS_PER_CORE = S // NCORES  # 1024
P = 128
M_TILES = ROWS_PER_CORE // P  # 8
K_TILES = D // P  # 8
N_CHUNK = 1024  # two PSUM banks per (m, chunk) tile
N_CHUNKS = S // N_CHUNK  # 8
N_HALF = 512  # max matmul moving free dim into one PSUM bank
INV_T = 10.0  # 1 / temperature
EPS = 1e-12
FP8_SCALE = 256.0  # input scale: keeps fp8e4m3 operands in their sweet spot
SIM_SCALE = INV_T / (FP8_SCALE * FP8_SCALE)  # exp(SIM_SCALE * raw - INV_T)

_FP32 = mybir.dt.float32
_FP8 = mybir.dt.float8e4
_BF16 = mybir.dt.bfloat16
_FP8_NP = mybir.dt.np(_FP8)


def _build_bass():
    # Bacc (not raw Bass): its compile() runs generate_event_semaphores,
    # which splits multi-semaphore waits into standalone EventSemaphore
    # instructions — engine instructions can encode only one wait.
    nc = bacc.Bacc("TRN2", debug=False, num_devices=NCORES, enable_partition_id=False)
    # lhsT blocked per m-tile on the host: [m, kt, p, col] so each m-block is
    # one contiguous 128KB DMA and the PE can ramp as soon as block 0 lands.
    lhsT = nc.dram_tensor(
        "lhst", [M_TILES, K_TILES, P, P], _FP8, kind="ExternalInput"
    ).ap()
    # brot blocked per 512-column half on the host: [half, p, kt, col] so
    # each partition reads 4KB contiguous runs per half-DMA.
    brot = nc.dram_tensor(
        "brot", [S // N_HALF, P, K_TILES, N_HALF], _FP8, kind="ExternalInput"
    ).ap()
    # Raw reductions out; the tiny final combine (a few K flops) runs on the
    # host, which avoids a 1.3us ACT table switch (Ln) in the device tail.
    sums_out = nc.dram_tensor(
        "sums", [P, M_TILES * N_CHUNKS], _FP32, kind="ExternalOutput"
    ).ap()
    diag_out = nc.dram_tensor(
        "diag", [P, 2 * M_TILES], _FP32, kind="ExternalOutput"
    ).ap()

    # Pre-TileContext const region (same pattern as Bass.__init__'s
    # const_aps): values read by hot-loop instructions with no tracked
    # dependency, so they add no per-instruction sync waits. Instead of a
    # full all-engine barrier (~3us butterfly), hand off with one semaphore
    # to the only consumers (ACT reads the bias const, DVE the identity).
    bias_th = nc.alloc_sbuf_tensor("const-f32-neg10", [P, 1], _FP32)
    nc.gpsimd.memset(bias_th.ap(), -INV_T)
    nc.const_aps.aps[(_FP32, -INV_T)] = bias_th.ap()
    ident_th = nc.alloc_sbuf_tensor("identity-f32", [P, P], _FP32)
    nc.gpsimd.memset(ident_th.ap(), 0.0)
    ident_inst = nc.gpsimd.affine_select(
        out=ident_th.ap(),
        in_=ident_th.ap(),
        compare_op=mybir.AluOpType.not_equal,
        fill=1.0,
        base=0,
        pattern=[[-1, P]],
        channel_multiplier=1,
    )
    const_sem = nc.alloc_semaphore("const-ready")
    ident_inst.then_inc(const_sem, 1)
    nc.vector.wait_ge(const_sem, 1)
    nc.scalar.wait_ge(const_sem, 1)

    with tile.TileContext(nc) as tc:
        _body(tc, lhsT, brot, sums_out, diag_out, ident_th.ap())
    nc.compile()
    return nc


def _body(tc, lhsT, brot, sums_out, diag_out, ident):
    nc = tc.nc
    AF = mybir.ActivationFunctionType

    # DRAM views with partition dim first: [p, kt, ...]
    a_view = lhsT.rearrange("m k p c -> p m k c")  # [128, 8, 8, 128]

    ctx = ExitStack()
    singles = ctx.enter_context(tc.tile_pool(name="singles", bufs=1))
    bpool = ctx.enter_context(tc.tile_pool(name="bchunks", bufs=3))
    # 4 tiles x 2 banks: deep PSUM pipeline so matmuls never wait on the
    # ACT exp/read-accumulator chain of the tile being recycled.
    pspool = ctx.enter_context(tc.tile_pool(name="psum", bufs=4, space="PSUM"))
    # Exp elementwise outputs are write-only garbage (the fused accum_out is
    # what we keep); bf16 halves their SBUF footprint.
    epool = ctx.enter_context(tc.tile_pool(name="exps", bufs=8))
    # Single-use slots for the 16 diagonal extractions.
    scratch = ctx.enter_context(tc.tile_pool(name="scratch", bufs=16))

    # Resident stationary operand: all local rows, transposed. SBUF layout
    # [p, kt, m*128+col]; m-block 0 is loaded before the first b chunk, the
    # rest right after it (the PE consumes m-blocks at ~2us each, so they
    # arrive well ahead).
    a_t = singles.tile([P, K_TILES, ROWS_PER_CORE], _FP8)

    def load_a_block(m):
        nc.sync.dma_start(
            out=a_t[:, :, m * P : (m + 1) * P], in_=a_view[:, m, :, :]
        )

    load_a_block(0)

    # Per-row partial sums: column m*N_CHUNKS + nch. Disjoint-column writes
    # carry no WAW dependencies between the exps.
    sums = singles.tile([P, M_TILES * N_CHUNKS], _FP32)
    # Raw (pre-exp, scaled) diagonal values: cols [0:8] positive, [8:16] self.
    diag = singles.tile([P, 2 * M_TILES], _FP32)

    for nch in range(N_CHUNKS):
        b_t = bpool.tile([P, K_TILES, N_CHUNK], _FP8)
        # Two half-loads (columns) so matmuls on the first PSUM bank can
        # start while the second half is still arriving.
        nc.sync.dma_start(out=b_t[:, :, 0:N_HALF], in_=brot[2 * nch])
        nc.sync.dma_start(out=b_t[:, :, N_HALF:N_CHUNK], in_=brot[2 * nch + 1])
        if nch == 0:
            for mb in range(1, M_TILES):
                load_a_block(mb)
        for m in range(M_TILES):
            ps = pspool.tile([P, N_CHUNK], _FP32)
            col = m * N_CHUNKS + nch
            for half in range(N_CHUNK // N_HALF):
                hs = slice(half * N_HALF, (half + 1) * N_HALF)
                for kt in range(0, K_TILES, 2):
                    nc.tensor.matmul(
                        ps[:, hs],
                        a_t[:, kt : kt + 2, m * P : (m + 1) * P],
                        b_t[:, kt : kt + 2, hs],
                        start=(kt == 0),
                        stop=(kt == K_TILES - 2),
                        perf_mode=mybir.MatmulPerfMode.DoubleRow,
                    )
            # exp over both PSUM banks at once; fused per-row accumulation.
            # All PE-group RAW waits share one semaphore.
            e_t = epool.tile([P, N_CHUNK], _BF16)
            nc.scalar.activation(
                out=e_t,
                in_=ps,
                func=AF.Exp,
                bias=-INV_T,
                scale=SIM_SCALE,
                accum_out=sums[:, col : col + 1],
            )
            # Diagonal extraction on the two special chunks. In rotated
            # coords, m-tile m's positive diagonal lives at columns
            # [m*128, (m+1)*128) -> chunk 0, offset 128*m; the self
            # diagonal at columns [4096 + m*128, ...) -> chunk 4.
            dcol = None
            if nch == 0:
                dcol = m
            elif nch == 4:
                dcol = M_TILES + m
            if dcol is not None:
                # DVE extracts the raw f32 diagonal straight from PSUM
                # (identity mul + reduce); Bacc's generate_event_semaphores
                # legalizes the resulting extra WAR wait on the recycling
                # matmul, and this keeps the ACT engine (the pipeline's
                # second-busiest) free of copy work.
                off = m * P
                diag_t = scratch.tile([P, P], _FP32)
                nc.vector.tensor_mul(diag_t, ps[:, off : off + P], ident)
                nc.vector.reduce_sum(
                    diag[:, dcol : dcol + 1], diag_t, axis=mybir.AxisListType.X
                )

        if nch == 4:
            # Both diagonals are complete; ship them while chunks 5-7 run.
            nc.sync.dma_start(out=diag_out, in_=diag)

    nc.sync.dma_start(out=sums_out, in_=sums)
    ctx.close()


_NC_CACHE = {}


def _get_nc():
    if "nc" not in _NC_CACHE:
        _NC_CACHE["nc"] = _build_bass()
    return _NC_CACHE["nc"]


def _make_in_maps(z1, z2):
    z1 = np.asarray(z1, dtype=np.float32)
    z2 = np.asarray(z2, dtype=np.float32)
    z = np.concatenate([z1, z2], axis=0)  # [8192, 1024]
    nrm = np.sqrt(np.sum(z * z, axis=1, keepdims=True, dtype=np.float32))
    n = z / np.maximum(nrm, EPS)
    repsT = np.ascontiguousarray(n.T * FP8_SCALE).astype(_FP8_NP)  # [1024, 8192]
    in_maps = []
    for c in range(NCORES):
        p0 = ((c * ROWS_PER_CORE) + B) % S
        rolled = np.concatenate([repsT[:, p0:], repsT[:, :p0]], axis=1)
        lhsT_c = repsT[:, c * ROWS_PER_CORE : (c + 1) * ROWS_PER_CORE]
        # Block per m-tile: [m, kt, p, col]
        lhsT_blk = np.ascontiguousarray(
            lhsT_c.reshape(K_TILES, P, M_TILES, P).transpose(2, 0, 1, 3)
        )
        # Block per 512-col half: [half, p, kt, col]
        b_blk = np.ascontiguousarray(
            rolled.reshape(K_TILES, P, S // N_HALF, N_HALF).transpose(2, 1, 0, 3)
        )
        in_maps.append({"lhst": lhsT_blk, "brot": b_blk})
    return in_maps


def _combine(results):
    # Per row i: T = S_full - e_self + e_pos; loss_row = ln(T) - (pos - 10)
    # with pos - 10 = SIM_SCALE*draw - 10. A few K flops; done in f64.
    total = 0.0
    for r in results:
        stot = r["sums"].astype(np.float64).reshape(P, M_TILES, N_CHUNKS).sum(axis=2)
        diag = r["diag"].astype(np.float64)
        draw, dself = diag[:, :M_TILES], diag[:, M_TILES:]
        e_pos = np.exp(SIM_SCALE * draw - INV_T)
        e_self = np.exp(SIM_SCALE * dself - INV_T)
        loss_rows = np.log(stot - e_self + e_pos) - (SIM_SCALE * draw - INV_T)
        total += float(loss_rows.sum())
    return np.array(total / S, dtype=np.float32)


def run_traced(z1, z2, **spmd_kwargs):
    """Run on HW with profiling; returns (loss, BassKernelResults)."""
    nc = _get_nc()
    in_maps = _make_in_maps(z1, z2)
    res = bass_utils.run_bass_kernel_spmd(
        nc, in_maps, core_ids=list(range(NCORES)), trace=True, **spmd_kwargs
    )
    return _combine(res.results), res


def kernel(z1, z2):
    nc = _get_nc()
    in_maps = _make_in_maps(z1, z2)
    last_err = None
    for _attempt in range(3):
        try:
            res = bass_utils.run_bass_kernel_spmd(
                nc, in_maps, core_ids=list(range(NCORES))
            )
            return _combine(res.results)
        except Exception as e:  # transient device wedge: retry
            last_err = e
            time.sleep(2.0)
    raise last_err



# revision 10
# speedup vs baseline: 1.4679x; 1.4679x over previous
"""Contrastive loss (SimCLR-style NT-Xent) Trainium2 kernel.

Full inputs z1, z2: [4096, 1024] f32. Output: scalar f32 loss.

Strategy (8 NeuronCores, SPMD, no collectives) — SYMMETRIC-TRIANGLE:
  sim = reps @ reps.T is symmetric, so only the upper triangle of the
  16x16 grid of 512x512 blocks is computed (136 blocks total, 17/core):
  each computed off-diagonal block (bi, bj) serves row-block bi via the
  ACT-fused per-row exp sums AND row-block bj via per-column exp sums
  (a cheap fp8 DoubleRow ones-matmul over the exp tile pairs). This
  nearly halves PE work vs the full row-sharded GEMM.

  - Host: L2-normalize rows of reps = concat(z1, z2) [8192, 1024] f32,
    transpose to repsT [1024, 8192], scale by 256, cast fp8e4m3.
  - Core c owns row-blocks {c, c+8} (512 rows each -> groups A, B). Its
    moving operand is repsT with columns rotated by c*512; in rotated
    coords every core computes the same block positions (SPMD):
      group A (rows c):   column chunks 0..8  (chunk 0 = self-diagonal
                          block; chunk 8 = block (c, c+8), whose local
                          diagonal holds the positive pairs)
      group B (rows c+8): column chunks 8..15 (chunk 8 = self-diagonal)
    Pair coverage: d = bj - bi mod 16 in {0..7} for every bi plus d=8
    for bi < 8 covers each unordered block pair exactly once.
  - Per (block, m-tile of 128 rows): 4 fp8 DoubleRow matmuls (K=1024)
    into one PSUM bank; ACT exp(s*x) (bias 0: off-diag values land in
    [e^-2, e^2], the fp8e4m3 sweet spot) with fused per-row accum_out;
    the exp tiles of off-diagonal blocks are written as fp8 pairs
    [128, 2, 512] and column-reduced with a DoubleRow ones-matmul into
    a PSUM row per chunk (the transposed-block row sums). Colsum
    matmuls are emitted one m-tile late so the in-order PE never waits
    on the ACT exp chain.
  - Diagonals (self A, positive, self B) extracted raw from PSUM on DVE
    (identity mul + reduce), as in the row-sharded baseline.
  - Host: assemble per-row totals T = rowsum - e_self + e_pos in f64,
    loss = mean(ln T - s*pos). A few K flops.
"""

import time
from collections import deque
from contextlib import ExitStack

import numpy as np
import ml_dtypes

import concourse.bass as bass
import concourse.tile as tile
from concourse import bacc
from concourse import mybir
from concourse import bass_utils

B = 4096
D = 1024
S = 2 * B            # 8192 rows/cols of sim
NCORES = 8
P = 128
BLK = 512            # block edge (= one PSUM bank of f32)
GRID = S // BLK      # 16
K_TILES = D // P     # 8
N_CHUNKS = GRID      # 16 column chunks of 512
INV_T = 10.0         # 1 / temperature
EPS = 1e-12
FP8_SCALE = 256.0    # input scale: keeps fp8e4m3 operands in their sweet spot
SIM_SCALE = INV_T / (FP8_SCALE * FP8_SCALE)  # exp(SIM_SCALE * raw)

_FP32 = mybir.dt.float32
_FP8 = mybir.dt.float8e4
_BF16 = mybir.dt.bfloat16
_FP8_NP = mybir.dt.np(_FP8)


def _build_bass():
    # Bacc (not raw Bass): its compile() runs generate_event_semaphores,
    # which splits multi-semaphore waits into standalone EventSemaphore
    # instructions — engine instructions can encode only one wait.
    nc = bacc.Bacc("TRN2", debug=False, num_devices=NCORES, enable_partition_id=False)
    # Stationary rows (A|B), blocked per kt-pair slab on the host:
    # [slab, p, j, m] so each partition reads 2KB contiguous per slab and
    # the PE can start after slab 0 + the first b piece land.
    a_in = nc.dram_tensor(
        "lhst", [K_TILES // 2, P, 2, 2 * BLK], _FP8, kind="ExternalInput"
    ).ap()
    # Rotated moving operand blocked per 512-col chunk: [ch, p, kt, col],
    # 4KB contiguous per partition per chunk.
    b_in = nc.dram_tensor(
        "brot", [N_CHUNKS, P, K_TILES, BLK], _FP8, kind="ExternalInput"
    ).ap()
    # Raw reductions out; the tiny final combine runs on the host.
    sums_out = nc.dram_tensor("sums", [P, 8, 9], _FP32, kind="ExternalOutput").ap()
    diag_out = nc.dram_tensor("diag", [P, 12], _FP32, kind="ExternalOutput").ap()
    cols_out = nc.dram_tensor("cols", [15, BLK], _FP32, kind="ExternalOutput").ap()
    # One-hot stationaries: colsum matmul for chunk ch routes its column
    # sums into row ch-1 of a single [16, 512] PSUM accumulator (matmul
    # output base partition must be 0), other rows accumulate exact zeros.
    oh_in = nc.dram_tensor(
        "onehot", [P, 15, 2, 16], _FP8, kind="ExternalInput"
    ).ap()

    # Pre-TileContext const region (same pattern as Bass.__init__'s
    # const_aps): values read by hot-loop instructions with no tracked
    # dependency, so they add no per-instruction sync waits. Hand off with
    # one semaphore to the consumers instead of a full barrier.
    bias_th = nc.alloc_sbuf_tensor("const-f32-zero", [P, 1], _FP32)
    nc.gpsimd.memset(bias_th.ap(), 0.0)
    nc.const_aps.aps[(_FP32, 0.0)] = bias_th.ap()
    ident_th = nc.alloc_sbuf_tensor("identity-f32", [P, P], _FP32)
    nc.gpsimd.memset(ident_th.ap(), 0.0)
    ident_inst = nc.gpsimd.affine_select(
        out=ident_th.ap(),
        in_=ident_th.ap(),
        compare_op=mybir.AluOpType.not_equal,
        fill=1.0,
        base=0,
        pattern=[[-1, P]],
        channel_multiplier=1,
    )
    const_sem = nc.alloc_semaphore("const-ready")
    ident_inst.then_inc(const_sem, 1)
    nc.vector.wait_ge(const_sem, 1)
    nc.scalar.wait_ge(const_sem, 1)

    with tile.TileContext(nc) as tc:
        _body(tc, a_in, b_in, oh_in, sums_out, diag_out, cols_out,
              ident_th.ap())
    nc.compile()
    return nc


def _body(tc, a_in, b_in, oh_in, sums_out, diag_out, cols_out, ident):
    nc = tc.nc
    AF = mybir.ActivationFunctionType

    ctx = ExitStack()
    singles = ctx.enter_context(tc.tile_pool(name="singles", bufs=1))
    bpool = ctx.enter_context(tc.tile_pool(name="bchunks", bufs=3))
    # 4 GEMM banks: deep PSUM pipeline so matmuls never wait on the ACT
    # exp/read-accumulator chain of the bank being recycled.
    pspool = ctx.enter_context(tc.tile_pool(name="psum", bufs=4, space="PSUM"))
    colpool = ctx.enter_context(tc.tile_pool(name="colps", bufs=1, space="PSUM"))
    # Off-diagonal exp pairs (fp8, feed the colsum matmuls).
    epool = ctx.enter_context(tc.tile_pool(name="exps", bufs=6))
    # Diagonal-block exp outputs are write-only garbage (self value e^10
    # would saturate fp8, so bf16), only accum_out is kept.
    edpool = ctx.enter_context(tc.tile_pool(name="expd", bufs=2))
    scratch = ctx.enter_context(tc.tile_pool(name="scratch", bufs=8))

    # Resident stationary operand: the core's 1024 rows (A|B) transposed,
    # [p, kt, m]; loaded as 4 kt-pair slabs so matmul (kt 0-1) starts as
    # soon as slab 0 and the first b piece land.
    a_t = singles.tile([P, K_TILES, 2 * BLK], _FP8)
    for s in range(4):
        nc.sync.dma_start(out=a_t[:, 2 * s : 2 * s + 2, :], in_=a_in[s])
    oh_t = singles.tile([P, 15, 2, 16], _FP8)
    nc.sync.dma_start(out=oh_t, in_=oh_in)

    # Per-row partial sums: [p, gmt, slot]; group A (gmt 0-3) slots 0..8
    # = chunks 0..8, group B (gmt 4-7) slots 0..7 = chunks 8..15.
    sums = singles.tile([P, 8, 9], _FP32)
    # Raw (pre-exp, scaled) diagonals: cols 0-3 self A, 4-7 positive,
    # 8-11 self B (by m-tile).
    diag = singles.tile([P, 12], _FP32)
    # Column sums (transposed-block row sums): row ch-1 <- chunk ch, all
    # 30 matmuls accumulate into one PSUM bank via one-hot stationaries.
    colps = colpool.tile([16, BLK], _FP32)
    colsb = singles.tile([15, BLK], _FP32)

    # Deferred colsum matmuls: emitted >=1 m-tile after their exp pair so
    # the in-order PE queue never stalls on ACT.
    pending = deque()
    mt_clock = [0]
    n_cols = [0]

    def flush_colsums(min_age):
        while pending and mt_clock[0] - pending[0][0] >= min_age:
            _, e_pair, ch = pending.popleft()
            n_cols[0] += 1
            nc.tensor.matmul(
                colps,
                oh_t[:, ch - 1],
                e_pair,
                start=(n_cols[0] == 1),
                stop=(n_cols[0] == 30),
                perf_mode=mybir.MatmulPerfMode.DoubleRow,
                skip_group_check=True,
            )

    for ch in range(N_CHUNKS):
        b_t = bpool.tile([P, K_TILES, BLK], _FP8)
        if ch == 0:
            # kt-pair pieces: matmul s waits only on its own 64KB piece.
            for s in range(4):
                nc.sync.dma_start(
                    out=b_t[:, 2 * s : 2 * s + 2, :], in_=b_in[0][:, 2 * s : 2 * s + 2, :]
                )
        else:
            nc.sync.dma_start(out=b_t, in_=b_in[ch])
        groups = (0,) if ch < 8 else ((0, 1) if ch == 8 else (1,))
        for g in groups:
            goff = g * BLK
            is_diag = (ch == 0 and g == 0) or (ch == 8 and g == 1)
            e_pair = None
            for mt in range(4):
                ps = pspool.tile([P, BLK], _FP32)
                for s in range(4):
                    nc.tensor.matmul(
                        ps,
                        a_t[:, 2 * s : 2 * s + 2, goff + mt * P : goff + (mt + 1) * P],
                        b_t[:, 2 * s : 2 * s + 2, :],
                        start=(s == 0),
                        stop=(s == 3),
                        perf_mode=mybir.MatmulPerfMode.DoubleRow,
                    )
                mt_clock[0] += 1
                flush_colsums(1)
                gmt = g * 4 + mt
                slot = ch if g == 0 else ch - 8
                if is_diag:
                    e_t = edpool.tile([P, BLK], _BF16)
                    nc.scalar.activation(
                        out=e_t, in_=ps, func=AF.Exp, bias=0.0, scale=SIM_SCALE,
                        accum_out=sums[:, gmt, slot : slot + 1],
                    )
                else:
                    if mt % 2 == 0:
                        e_pair = epool.tile([P, 2, BLK], _FP8)
                    nc.scalar.activation(
                        out=e_pair[:, mt % 2, :], in_=ps, func=AF.Exp,
                        bias=0.0, scale=SIM_SCALE,
                        accum_out=sums[:, gmt, slot : slot + 1],
                    )
                    if mt % 2 == 1:
                        pending.append((mt_clock[0], e_pair, ch))
                # Raw diagonal extraction on DVE straight from PSUM: the
                # self diagonals (diag blocks) and the positive diagonal
                # (block (c, c+8) = chunk 8, group A).
                dcol = None
                if is_diag:
                    dcol = (0 if ch == 0 else 8) + mt
                elif ch == 8 and g == 0:
                    dcol = 4 + mt
                if dcol is not None:
                    off = mt * P
                    diag_t = scratch.tile([P, P], _FP32)
                    nc.vector.tensor_mul(diag_t, ps[:, off : off + P], ident)
                    nc.vector.reduce_sum(
                        diag[:, dcol : dcol + 1], diag_t, axis=mybir.AxisListType.X
                    )
        if ch == 8:
            # All three diagonals are complete; ship while chunks 9-15 run.
            nc.sync.dma_start(out=diag_out, in_=diag)

    flush_colsums(0)
    # Drain colsums PSUM -> SBUF -> DRAM (ACT copy keeps partitions aligned).
    nc.scalar.activation(out=colsb, in_=colps[0:15, :], func=AF.Copy)
    nc.sync.dma_start(out=cols_out, in_=colsb)
    nc.sync.dma_start(out=sums_out, in_=sums)
    ctx.close()


_NC_CACHE = {}


def _get_nc():
    if "nc" not in _NC_CACHE:
        _NC_CACHE["nc"] = _build_bass()
    return _NC_CACHE["nc"]


def _make_in_maps(z1, z2):
    z1 = np.asarray(z1, dtype=np.float32)
    z2 = np.asarray(z2, dtype=np.float32)
    z = np.concatenate([z1, z2], axis=0)  # [8192, 1024]
    nrm = np.sqrt(np.sum(z * z, axis=1, keepdims=True, dtype=np.float32))
    n = z / np.maximum(nrm, EPS)
    repsT = np.ascontiguousarray(n.T * FP8_SCALE).astype(_FP8_NP)  # [1024, 8192]
    in_maps = []
    for c in range(NCORES):
        rolled = np.concatenate([repsT[:, c * BLK :], repsT[:, : c * BLK]], axis=1)
        aT = np.concatenate(
            [repsT[:, c * BLK : (c + 1) * BLK],
             repsT[:, (c + 8) * BLK : (c + 9) * BLK]], axis=1)  # [1024, 1024]
        # kt-pair slabs: [slab, p, j, m]
        a_blk = np.ascontiguousarray(
            aT.reshape(4, 2, P, 2 * BLK).transpose(0, 2, 1, 3))
        # per-chunk: [ch, p, kt, col]
        b_blk = np.ascontiguousarray(
            rolled.reshape(K_TILES, P, N_CHUNKS, BLK).transpose(2, 1, 0, 3))
        in_maps.append({"lhst": a_blk, "brot": b_blk, "onehot": _onehot()})
    return in_maps


def _onehot():
    oh = np.zeros((P, 15, 2, 16), dtype=_FP8_NP)
    for s in range(15):
        oh[:, s, :, s] = _FP8_NP(1.0)
    return oh


def _combine(results):
    # Per row i: T = rowsum - e_self + e_pos; loss_row = ln(T) - s*pos.
    # A few K flops; done in f64.
    rowsum = np.zeros(S, dtype=np.float64)
    selfraw = np.zeros(S, dtype=np.float64)
    posraw = np.zeros(S, dtype=np.float64)
    p = np.arange(P)
    for c, r in enumerate(results):
        sums = r["sums"].astype(np.float64)  # [128, 8, 9]
        diag = r["diag"].astype(np.float64)  # [128, 12]
        cols = r["cols"].astype(np.float64)  # [15, 512]
        for mt in range(4):
            rA = c * BLK + mt * P + p
            rB = (c + 8) * BLK + mt * P + p
            rowsum[rA] += sums[:, mt, 0:9].sum(axis=1)
            rowsum[rB] += sums[:, 4 + mt, 0:8].sum(axis=1)
            selfraw[rA] = diag[:, mt]
            posraw[rA] = diag[:, 4 + mt]
            posraw[rA + B] = diag[:, 4 + mt]
            selfraw[rB] = diag[:, 8 + mt]
        for ch in range(1, 16):
            tb = (c + ch) % GRID
            rowsum[tb * BLK : (tb + 1) * BLK] += cols[ch - 1]
    T = rowsum - np.exp(SIM_SCALE * selfraw) + np.exp(SIM_SCALE * posraw)
    loss_rows = np.log(T) - SIM_SCALE * posraw
    return np.array(loss_rows.mean(), dtype=np.float32)


def run_traced(z1, z2, **spmd_kwargs):
    """Run on HW with profiling; returns (loss, BassKernelResults)."""
    nc = _get_nc()
    in_maps = _make_in_maps(z1, z2)
    res = bass_utils.run_bass_kernel_spmd(
        nc, in_maps, core_ids=list(range(NCORES)), trace=True, **spmd_kwargs
    )
    return _combine(res.results), res


def kernel(z1, z2):
    nc = _get_nc()
    in_maps = _make_in_maps(z1, z2)
    last_err = None
    for _attempt in range(3):
        try:
            res = bass_utils.run_bass_kernel_spmd(
                nc, in_maps, core_ids=list(range(NCORES))
            )
            return _combine(res.results)
        except Exception as e:  # transient device wedge: retry
            last_err = e
            time.sleep(2.0)
    raise last_err
